# revision 15
# baseline (speedup 1.0000x reference)
"""Bicubic warp-interpolator Trainium2 kernel.

Math (per channel-image, reference semantics):
    x_map = ((x + dx - W/2)/(W/2-1) + 1) * (W-1)/2          per-pixel
    y_map likewise;  x0 = floor(x_map), tx = frac, 16 Catmull-Rom taps at
    clip(x0-1+i), clip(y0-1+j) with cubic weights wx_i(tx), wy_j(ty).

Kernel strategy ("scheme V"):
    The vertical 4-tap cubic is a polynomial in ty:
        sum_j wy_j(ty) img[clip(Y+j)] = sum_m ty^m V_m[Y]
    where V_m[r, c] = sum_j A[j][m] img[clip(r-8+j), clip(c-8)] are FIXED
    4-tap vertical convolutions -> computed on the TensorEngine as banded
    matmuls and stored interleaved in DRAM as V4[r, c, m] (m fastest).
    Then every output pixel needs exactly the 16 contiguous floats
    V4[r, c:c+4, 0:4] -> ONE 64-byte indirect-DMA descriptor per pixel.
    DVE computes coords/weights and contracts the gathered 16-vector with
    wx_i * ty^m.

Sharding: pure data parallel, one batch image (3 channels) per core.
"""

import sys

for _p in ("/opt/trn_rl_repo", "/root/.axon_site/_ro/trn_rl_repo"):
    if _p not in sys.path:
        sys.path.insert(0, _p)

import numpy as np

import concourse.bass as bass
import concourse.bacc as bacc
import concourse.mybir as mybir
import concourse.tile as tile

F32 = mybir.dt.float32
I32 = mybir.dt.int32
AL = mybir.AluOpType
AF = mybir.ActivationFunctionType

# Catmull-Rom tap-j weight = sum_m A[j][m] * t^m
A_COEF = np.array(
    [
        [0.0, -0.5, 1.0, -0.5],
        [1.0, 0.0, -2.5, 1.5],
        [0.0, 0.5, 2.0, -1.5],
        [0.0, 0.0, -0.5, 0.5],
    ],
    dtype=np.float64,
)

PAD = 8  # window padding: tap base index r = floor(map) - 1 + PAD >= 0


def host_constants(H: int, W: int, nband_rows: int):
    """Data-independent constant tensors shipped as extra kernel inputs."""
    # x_rel = alpha * dx + xcon[x];   x_rel = x_map + PAD
    ax = ((W - 1) / 2.0) / (W / 2.0 - 1.0)
    ay = ((H - 1) / 2.0) / (H / 2.0 - 1.0)
    x = np.arange(W, dtype=np.float64)
    y = np.arange(H, dtype=np.float64)
    xcon = ((x - W / 2.0) / (W / 2.0 - 1.0) + 1.0) * ((W - 1) / 2.0) + PAD
    ycon = ((y - H / 2.0) / (H / 2.0 - 1.0) + 1.0) * ((H - 1) / 2.0) + PAD
    xcon_t = np.tile(xcon.astype(np.float32)[None, :], (nband_rows, 1))
    ycon_t = ycon.astype(np.float32)[:, None]

    return {
        "xcon": xcon_t,
        "ycon": ycon_t,
        "alpha_x": float(np.float32(ax)),
        "alpha_y": float(np.float32(ay)),
    }


def block_geometry(H: int, R4: int):
    """Stage-1 block list: (R0, Mb, lo, hi) with clip folded into lhsT."""
    blocks = []
    n_blocks = (R4 + 124) // 125
    for b in range(n_blocks):
        R0 = 125 * b
        Mb = min(125, R4 - R0)
        s0 = R0 - PAD
        lo = max(0, s0)
        hi = min(H, s0 + Mb + 3)
        blocks.append((R0, Mb, lo, hi))
    return blocks


def band_variants(H: int, R4: int):
    """Per-block banded lhsT with border clamping folded in; deduped.

    Returns (band_concat [128, nvar*4*128] f32, per-block variant index).
    variant v, plane m lives at cols v*512 + m*128 + [0, Mb).
    lhsT[k, q] = sum_j A[j][m] [clip(R0+q-PAD+j) == lo+k]
    """
    mats = []
    keys = {}
    vidx = []
    for (R0, Mb, lo, hi) in block_geometry(H, R4):
        Ksrc = hi - lo
        M = np.zeros((128, 4 * 128), dtype=np.float32)
        for q in range(Mb):
            r = R0 + q
            for j in range(4):
                t = min(max(r - PAD + j, 0), H - 1)
                k = t - lo
                if 0 <= k < Ksrc:
                    for m in range(4):
                        M[k, m * 128 + q] += A_COEF[j][m]
        key = M.tobytes()
        if key not in keys:
            keys[key] = len(mats)
            mats.append(M)
        vidx.append(keys[key])
    return np.concatenate(mats, axis=1), vidx


def build_program(
    nimg: int,
    H: int,
    W: int,
    CH: int = 256,
    prod_gpsimd_mod: int = 0,  # every k-th chunk's big ops go to gpsimd (0=never)
    dbg_v4: bool = False,
    sg_for_band=None,  # optional: band index -> pixels per partition per gather call
):
    """Build the SPMD Bass program for one core handling `nimg` HxW images."""
    R4 = H + 2 * PAD - 2  # tap-base rows r in [0, H+12]; r_max = H+12-1... safe bound
    # r = floor(y_map) - 1 + PAD; y_map in (-PAD+1, H-1+PAD-2ish) -> r in [0, H+2*PAD-3]
    R4 = H + 2 * PAD  # a touch of slack
    C4 = W + 2 * PAD  # c in [0, W+2*PAD-3], runs c..c+3 -> C4 >= W+2*PAD-... ok
    BAND = min(128, H)
    n_bands = (H + BAND - 1) // BAND
    n_chunks = (W + CH - 1) // CH
    MMN = 512  # matmul free-dim chunk

    cst = host_constants(H, W, BAND)
    band_arr, vidx = band_variants(H, R4)
    cst["band"] = band_arr

    nc = bacc.Bacc("TRN2", target_bir_lowering=False, debug=False, num_devices=8)
    img_p = nc.declare_dram_parameter("img", [nimg, H, W], F32, isOutput=False)
    dx_p = nc.declare_dram_parameter("dx", [nimg, H, W], F32, isOutput=False)
    dy_p = nc.declare_dram_parameter("dy", [nimg, H, W], F32, isOutput=False)
    xcon_p = nc.declare_dram_parameter("xcon", [BAND, W], F32, isOutput=False)
    ycon_p = nc.declare_dram_parameter("ycon", [H, 1], F32, isOutput=False)
    band_p = nc.declare_dram_parameter(
        "band", [128, band_arr.shape[1]], F32, isOutput=False
    )
    out_p = nc.declare_dram_parameter("out", [nimg, H, W], F32, isOutput=True)
    NU_ = (H + 2 * PAD + 1) * C4 + 2
    v4dbg_p = (
        nc.declare_dram_parameter("v4dbg", [NU_, 4], F32, isOutput=True)
        if dbg_v4
        else None
    )

    with tile.TileContext(nc) as tc:
        with (
            tc.tile_pool(name="dram", bufs=2, space="DRAM") as dramp,
            tc.tile_pool(name="consts", bufs=1) as cstp,
            tc.tile_pool(name="s1src", bufs=2) as s1src,
            tc.tile_pool(name="s1v4", bufs=2) as s1v4,
            tc.tile_pool(name="psum", bufs=4, space="PSUM") as psump,
            tc.tile_pool(name="dxy", bufs=2) as dxyp,
            tc.tile_pool(name="small", bufs=1) as smallp,
            tc.tile_pool(name="pipe2", bufs=3) as pipe2p,
            tc.tile_pool(name="big", bufs=2) as bigp,
            tc.tile_pool(name="c16p", bufs=2) as c16p,
            tc.tile_pool(name="obnd", bufs=2) as obndp,
        ):
            # ---- persistent constants in SBUF ----
            urep_i = cstp.tile([BAND, 4], I32)
            nc.gpsimd.iota(urep_i[:], pattern=[[1, 4]], base=0, channel_multiplier=0)
            urep = cstp.tile([BAND, 4], F32)
            nc.vector.tensor_copy(urep[:], urep_i[:])
            xcon_sb = cstp.tile([BAND, W], F32)
            nc.sync.dma_start(out=xcon_sb[:], in_=xcon_p[:])
            band_sb = cstp.tile([128, band_arr.shape[1]], F32)
            nc.sync.dma_start(out=band_sb[:], in_=band_p[:])
            ycon_sb = cstp.tile([H, 1] if H <= 128 else [128, (H + 127) // 128], F32)
            if H <= 128:
                nc.sync.dma_start(out=ycon_sb[:], in_=ycon_p[:])
            else:
                # [H,1] -> [128, H/128] with column b holding rows b*128..b*128+127
                nc.sync.dma_start(
                    out=ycon_sb[:],
                    in_=ycon_p[:].rearrange("(b p) o -> p (b o)", p=128),
                )

            blocks = block_geometry(H, R4)
            nblk = len(blocks)
            NU = (R4 + 1) * C4 + 2

            def emit_stage1_block(im, v4, b):
                # ===== stage 1 block: V4 planes via PE band matmuls =====
                # Layout: unit u = (r+1)*C4 + (c+1) holds V4[r, c, 0:4], so the
                # gather index iy*C4 + ix needs no constant bias (the ISA's
                # indirect-DMA offset field can't hold a negative constant).
                R0, Mb, lo, hi = blocks[b]
                Ksrc = hi - lo
                var = vidx[b]
                src = s1src.tile([128, W], F32, name="src", tag="src")
                nc.sync.dma_start(out=src[0:Ksrc, :], in_=img_p[im, lo:hi, :])
                v4sb = s1v4.tile([128, C4 * 4], F32, name="v4sb", tag="v4sb")
                v4sb4 = v4sb[:].rearrange("p (c m) -> p c m", m=4)
                for m in range(4):
                    for h0 in range(0, W, MMN):
                        hn = min(MMN, W - h0)
                        ps = psump.tile([128, MMN], F32, space="PSUM")
                        nc.tensor.matmul(
                            out=ps[0:Mb, 0:hn],
                            lhsT=band_sb[
                                0:Ksrc, var * 512 + m * 128 : var * 512 + m * 128 + Mb
                            ],
                            rhs=src[0:Ksrc, h0 : h0 + hn],
                            start=True,
                            stop=True,
                        )
                        nc.scalar.copy(
                            out=v4sb4[0:Mb, PAD + h0 : PAD + h0 + hn, m],
                            in_=ps[0:Mb, 0:hn],
                        )
                    # replicate-pad left/right columns
                    nc.scalar.copy(
                        out=v4sb4[0:Mb, 0:PAD, m],
                        in_=v4sb4[0:Mb, PAD : PAD + 1, m].to_broadcast([Mb, PAD]),
                    )
                    nc.scalar.copy(
                        out=v4sb4[0:Mb, PAD + W : C4, m],
                        in_=v4sb4[0:Mb, PAD + W - 1 : PAD + W, m].to_broadcast(
                            [Mb, C4 - PAD - W]
                        ),
                    )
                u0 = (R0 + 1) * C4 + 1
                nc.sync.dma_start(
                    out=v4[u0 : u0 + Mb * C4, :].rearrange(
                        "(r c) m -> r (c m)", c=C4
                    ),
                    in_=v4sb[0:Mb, :],
                )

            # stage 1 of image 0 up-front; image im+1's stage 1 is interleaved
            # into image im's band loop so the gather stream never starves at
            # image transitions.
            v4_next = (
                v4dbg_p if dbg_v4 else dramp.tile([NU, 4], F32, name="v4", tag="v4")
            )
            for b in range(nblk):
                emit_stage1_block(0, v4_next, b)

            for im in range(nimg):
                v4 = v4_next
                if im + 1 < nimg:
                    v4_next = dramp.tile([NU, 4], F32, name="v4", tag="v4")
                else:
                    v4_next = None

                # ============ stage 2: per-band gather + combine ============
                v4flat = v4[:]
                for bd in range(n_bands):
                    if v4_next is not None:
                        for b in range(
                            (bd * nblk) // n_bands, ((bd + 1) * nblk) // n_bands
                        ):
                            emit_stage1_block(im + 1, v4_next, b)
                    Y0 = bd * BAND
                    rows = min(BAND, H - Y0)
                    dxt = dxyp.tile([BAND, W], F32, name="dxt", tag="dxt")
                    dyt = dxyp.tile([BAND, W], F32, name="dyt", tag="dyt")
                    nc.sync.dma_start(out=dxt[0:rows, :], in_=dx_p[im, Y0 : Y0 + rows, :])
                    nc.sync.dma_start(out=dyt[0:rows, :], in_=dy_p[im, Y0 : Y0 + rows, :])
                    if H <= 128:
                        ycon_ap = ycon_sb[Y0 : Y0 + rows, 0:1]
                    else:
                        ycon_ap = ycon_sb[0:rows, bd : bd + 1]
                    oband = obndp.tile([BAND, W], F32)

                    for ci in range(n_chunks):
                        c0 = ci * CH
                        cw = min(CH, W - c0)
                        sl = slice(c0, c0 + cw)
                        heavy_eng = nc.vector
                        if prod_gpsimd_mod and (ci % prod_gpsimd_mod == 0):
                            heavy_eng = nc.gpsimd

                        def st(name, dt=F32, n=cw):
                            return smallp.tile([BAND, n], dt, name=name, tag=name)[0:rows, :]

                        xr = st("xr")
                        nc.vector.scalar_tensor_tensor(
                            xr, dxt[0:rows, sl], cst["alpha_x"], xcon_sb[0:rows, sl],
                            AL.mult, AL.add,
                        )
                        yr = st("yr")
                        nc.vector.tensor_scalar(
                            yr, dyt[0:rows, sl], cst["alpha_y"], ycon_ap,
                            AL.mult, AL.add,
                        )
                        ix = st("ix", I32)
                        nc.vector.tensor_copy(ix, xr)
                        iy = st("iy", I32)
                        nc.vector.tensor_copy(iy, yr)
                        ixf = st("ixf")
                        nc.scalar.copy(out=ixf, in_=ix)
                        iyf = st("iyf")
                        nc.scalar.copy(out=iyf, in_=iy)
                        tx = st("tx")
                        nc.vector.tensor_tensor(out=tx, in0=xr, in1=ixf, op=AL.subtract)
                        ty = st("ty")
                        nc.vector.tensor_tensor(out=ty, in0=yr, in1=iyf, op=AL.subtract)
                        # HW f32->i32 cast rounds-to-nearest (CoreSim truncates);
                        # normalize to floor: where frac < 0, shift down by one.
                        mx = st("mx")
                        nc.vector.tensor_scalar(mx, tx, 0.0, None, AL.is_lt)
                        my = st("my")
                        nc.vector.tensor_scalar(my, ty, 0.0, None, AL.is_lt)
                        ixf2 = st("ixf2")
                        nc.vector.tensor_tensor(out=ixf2, in0=ixf, in1=mx, op=AL.subtract)
                        iyf2 = st("iyf2")
                        nc.vector.tensor_tensor(out=iyf2, in0=iyf, in1=my, op=AL.subtract)
                        txc = st("txc")
                        nc.vector.tensor_tensor(out=txc, in0=tx, in1=mx, op=AL.add)
                        tyc = st("tyc")
                        nc.vector.tensor_tensor(out=tyc, in0=ty, in1=my, op=AL.add)
                        ixf, iyf, tx, ty = ixf2, iyf2, txc, tyc

                        # gather index = (iy-1)*C4 + (ix-1) ; unit = 4 floats
                        idxf = st("idxf")
                        nc.vector.scalar_tensor_tensor(
                            idxf, iyf, float(C4), ixf, AL.mult, AL.add
                        )
                        # HW SWDGE consumes one offset per gathered unit-row
                        # (4 per pixel), so expand idx -> idx+u for u in 0..3;
                        # with that filling, HW (reads every 4th) and CoreSim
                        # (gathers 4 floats per offset) agree exactly.
                        idx4f = st("idx4f", n=cw * 4)
                        nc.vector.tensor_tensor(
                            out=idx4f.rearrange("p (n u) -> p n u", u=4),
                            in0=idxf.to_broadcast([rows, cw, 4]),
                            in1=urep[0:rows, :]
                            .rearrange("p (o u) -> p o u", o=1)
                            .broadcast_to([rows, cw, 4]),
                            op=AL.add,
                        )
                        idx = pipe2p.tile([BAND, CH * 4], I32, name="idx", tag="idx")[
                            0:rows, 0 : cw * 4
                        ]
                        nc.vector.tensor_copy(idx, idx4f)

                        # HW SWDGE emits ONE descriptor per contiguous partition-row
                        # run of the out AP, consuming one offset each -> SG=1 for
                        # a flat 2D out. Mode "ap3" makes each 64B unit its own AP
                        # row instead, so one call can carry many offsets.
                        mode = ("stripe", 8) if sg_for_band is None else sg_for_band(bd)
                        if mode[0] != "stripe":
                            gat = bigp.tile([BAND, CH * 16], F32, name="gat", tag="gat")[0:rows, 0 : cw * 16]
                        if mode[0] == "stripe":
                            # Round-robin the 64B gathers over NS stripe tiles so
                            # consecutive calls have no tile dependency and several
                            # DMAs stay in flight (hide the ~1us completion latency).
                            NS = mode[1]
                            cq = cw // NS
                            gstr = [
                                bigp.tile(
                                    [BAND, (CH // NS) * 16], F32,
                                    name=f"gs{k}", tag=f"gs{k}",
                                )[0:rows, 0 : cq * 16]
                                for k in range(NS)
                            ]
                            for s0 in range(0, cw):
                                k = s0 % NS
                                j = s0 // NS
                                nc.gpsimd.indirect_dma_start(
                                    out=gstr[k][:, j * 16 : (j + 1) * 16],
                                    out_offset=None,
                                    in_=v4flat,
                                    in_offset=bass.IndirectOffsetOnAxis(
                                        ap=idx[:, s0 * 4 : (s0 + 1) * 4], axis=0
                                    ),
                                )
                        elif mode[0] == "sg":
                            SG = mode[1]
                            for s0 in range(0, cw, SG):
                                sn = min(SG, cw - s0)
                                nc.gpsimd.indirect_dma_start(
                                    out=gat[:, s0 * 16 : (s0 + sn) * 16],
                                    out_offset=None,
                                    in_=v4flat,
                                    in_offset=bass.IndirectOffsetOnAxis(
                                        ap=idx[:, s0 * 4 : (s0 + sn) * 4], axis=0
                                    ),
                                )
                        else:
                            _, SG, expand = mode
                            for s0 in range(0, cw, SG):
                                sn = min(SG, cw - s0)
                                off_ap = idx[:, s0 * 4 : (s0 + sn) * 4]
                                nc.gpsimd.indirect_dma_start(
                                    out=gat[:, s0 * 16 : (s0 + sn) * 16].rearrange(
                                        "p (n u) -> p n u", u=16
                                    ),
                                    out_offset=None,
                                    in_=v4flat,
                                    in_offset=bass.IndirectOffsetOnAxis(
                                        ap=off_ap, axis=0
                                    ),
                                )

                        # powers & weights
                        tx2 = st("tx2")
                        nc.scalar.square(tx2, tx)
                        ty2 = st("ty2")
                        nc.scalar.square(ty2, ty)
                        tx3 = st("tx3")
                        nc.vector.tensor_tensor(out=tx3, in0=tx2, in1=tx, op=AL.mult)
                        ty3 = st("ty3")
                        nc.vector.tensor_tensor(out=ty3, in0=ty2, in1=ty, op=AL.mult)

                        u0 = st("u0")
                        nc.vector.scalar_tensor_tensor(u0, tx3, -0.5, tx2, AL.mult, AL.add)
                        wx0 = st("wx0")
                        nc.vector.scalar_tensor_tensor(wx0, tx, -0.5, u0, AL.mult, AL.add)
                        u1 = st("u1")
                        nc.vector.tensor_scalar(u1, tx2, -2.5, 1.0, AL.mult, AL.add)
                        wx1 = st("wx1")
                        nc.vector.scalar_tensor_tensor(wx1, tx3, 1.5, u1, AL.mult, AL.add)
                        u2 = st("u2")
                        nc.vector.tensor_scalar(u2, tx3, -1.5, None, AL.mult)
                        u2b = st("u2b")
                        nc.vector.scalar_tensor_tensor(u2b, tx2, 2.0, u2, AL.mult, AL.add)
                        wx2 = st("wx2")
                        nc.vector.scalar_tensor_tensor(wx2, tx, 0.5, u2b, AL.mult, AL.add)
                        u3 = st("u3")
                        nc.vector.tensor_scalar(u3, tx, 1.0, 0.5, AL.subtract, AL.mult)
                        wx3 = st("wx3")
                        nc.vector.tensor_tensor(out=wx3, in0=u3, in1=tx2, op=AL.mult)

                        # coefficient tile C16[p, n, i, m] = wx_i * ty^m
                        c16 = c16p.tile([BAND, CH * 16], F32, name="c16", tag="c16")
                        c16v = c16[:].rearrange("p (n i m) -> p n i m", i=4, m=4)
                        wxs = (wx0, wx1, wx2, wx3)
                        tys = (None, ty, ty2, ty3)
                        for i in range(4):
                            nc.scalar.copy(out=c16v[0:rows, 0:cw, i, 0], in_=wxs[i])
                            for m in range(1, 4):
                                heavy_eng.tensor_tensor(
                                    out=c16v[0:rows, 0:cw, i, m],
                                    in0=wxs[i],
                                    in1=tys[m],
                                    op=AL.mult,
                                )

                        red = st("red")
                        if mode[0] == "stripe":
                            NS = mode[1]
                            cq = cw // NS
                            c16str = c16[0:rows, 0 : cw * 16].rearrange(
                                "p (j k s) -> p j k s", k=NS, s=16
                            )
                            redv = red.rearrange("p (j k) -> p j k", k=NS)
                            for k in range(NS):
                                prodk = bigp.tile(
                                    [BAND, (CH // NS) * 16], F32,
                                    name="pr", tag="pr",
                                )[0:rows, 0 : cq * 16]
                                heavy_eng.tensor_tensor(
                                    out=prodk,
                                    in0=gstr[k],
                                    in1=c16str[:, :, k, :],
                                    op=AL.mult,
                                )
                                nc.vector.tensor_reduce(
                                    redv[:, :, k],
                                    prodk.rearrange("p (n s) -> p n s", s=16),
                                    mybir.AxisListType.X,
                                    AL.add,
                                )
                        else:
                            prod = bigp.tile([BAND, CH * 16], F32, name="prod", tag="prod")[
                                0:rows, 0 : cw * 16
                            ]
                            heavy_eng.tensor_tensor(
                                out=prod, in0=gat, in1=c16[0:rows, 0 : cw * 16], op=AL.mult
                            )
                            nc.vector.tensor_reduce(
                                red,
                                prod.rearrange("p (n s) -> p n s", s=16),
                                mybir.AxisListType.X,
                                AL.add,
                            )
                        nc.vector.tensor_scalar(
                            oband[0:rows, sl], red, 1.0, 0.0, AL.min, AL.max
                        )
                    nc.sync.dma_start(
                        out=out_p[im, Y0 : Y0 + rows, :], in_=oband[0:rows, :]
                    )
    nc.compile()
    return nc, cst


def kernel(input_image: np.ndarray, delta_x: np.ndarray, delta_y: np.ndarray):
    """Full-input entry point: shards batch across 8 NeuronCores."""
    from concourse.bass_utils import run_bass_kernel_spmd

    B, C, H, W = input_image.shape
    n_cores = 8
    assert B % n_cores == 0
    per = B // n_cores  # images (batch entries) per core
    nimg = per * C

    nc, cst = build_program(nimg, H, W)

    in_maps = []
    for c in range(n_cores):
        sl = slice(c * per, (c + 1) * per)
        in_maps.append(
            {
                "img": np.ascontiguousarray(
                    input_image[sl].reshape(nimg, H, W)
                ).astype(np.float32),
                "dx": np.ascontiguousarray(delta_x[sl].reshape(nimg, H, W)).astype(
                    np.float32
                ),
                "dy": np.ascontiguousarray(delta_y[sl].reshape(nimg, H, W)).astype(
                    np.float32
                ),
                "xcon": cst["xcon"],
                "ycon": cst["ycon"],
                "band": cst["band"],
            }
        )
    res = run_bass_kernel_spmd(nc, in_maps, list(range(n_cores)))
    out = np.empty((B, C, H, W), dtype=np.float32)
    for c in range(n_cores):
        out[c * per : (c + 1) * per] = res.results[c]["out"].reshape(per, C, H, W)
    return out


if __name__ == "__main__":
    pass

